# revision 20
# baseline (speedup 1.0000x reference)
"""MultiHeadAttention (B=2, S=2048, D=1024, H=16) on 8 TRN2 NeuronCores.

Sharding: core c -> batch b = c//4, head-group g = c%4 (4 heads = 256 channels).
Each core computes its 4 heads' attention for its batch plus the partial
out-projection (out_w columns for its channel group); host sums the 4 partials
per batch and adds out_b.

v2 design (from NTFF trace analysis of the v1 baseline, 304-362us):
 - At warm clock (2.4 GHz) the attention inner loop is ACT-bound, not
   PE-bound: each [128,512] exp costs (512+352)/1.2 ns -- a 352-cycle fixed
   overhead per ACTIVATE -- and ACT ran 89% busy in the warm stretch while
   PE matmuls have slack. So:
     * exps are batched: ONE ACTIVATE per key-chunk over a [128,1024] PSUM
       tile (two banks, both heads' logits side by side) -> halves the
       per-instruction overhead on the bottleneck engine.
     * phase A is folded into the attention phase: only KT-jc0, V, and
       QT-jc0-qt0 are computed up front (~22us instead of ~49us serial);
       the remaining Q/K projection matmuls stream into the ACT-bound kc
       loop as PE fill-in (2 matmuls per kc slot), loop is pair-outer so
       jc1 projections are only needed after pair 0 completes.
     * out-projection units fill the pair-1 kc slots the same way.
 - Denominators: ones column per head in Vaug (index HD within each VW=65
   group) makes softmax denominators fall out of the AV matmul; each head's
   denominator row is broadcast with its own K=1 ones-matmul (no SBUF->SBUF
   DMA scatter hop), reciprocal on DVE, per-row tensor_mul normalize.
 - av PSUM banks run bufs=1 (8-bank budget: lg 2x2 + av0 + av1 + op 2x2);
   the av rows are copied to SBUF (fp32, full precision) right after the
   last AV matmul so the WAR for the next block clears early.
 - Dtypes: all matmul operands 2-byte (fp16; exp outputs / denominator rows
   bf16 for range -- logits reach ~50 so e^50 overflows fp16). fp32 PE
   matmuls draw the DEC throttle to its lowest p-state; fp8 measured at
   exactly fp16 speed (duty-based clamp, dtype-blind below 2 bytes).
   Accumulation fp32 in PSUM.
 - Timing is thermally sensitive (~60us swings back-to-back); compare runs
   only after >=150s idle.
"""

import os
import sys

import numpy as np

for _p in ("/opt/trn_rl_repo",):
    if os.path.isdir(_p) and _p not in sys.path:
        sys.path.insert(0, _p)

from collections import deque
from contextlib import ExitStack

import concourse.bass as bass
import concourse.tile as tile
from concourse import bacc, mybir
from concourse._compat import with_exitstack
from concourse.bass_utils import run_bass_kernel_spmd

B, S, D = 2, 2048, 1024
H = 16
HD = 64
NCORES = 8
JG = 256          # channels per core (4 heads)
DC = D // 128     # 8 contraction chunks
QT_TILES = 4      # 4 x 512 query tiles
KC = S // 128     # 16 key chunks
VW = 65           # V columns per head incl. ones column
FP32 = mybir.dt.float32
FP16 = mybir.dt.float16
BF16 = mybir.dt.bfloat16
EXP = mybir.ActivationFunctionType.Exp


@with_exitstack
def mha_core_kernel(ctx: ExitStack, tc: tile.TileContext,
                    out, xT, wqT, wkT, wvT, bq, bk, bv, owT):
    nc = tc.nc
    ctx.enter_context(nc.allow_low_precision("2-byte matmul operands"))

    persist = ctx.enter_context(tc.tile_pool(name="persist", bufs=1))
    QT_sb = persist.tile((128, 2 * S), FP16)
    KT_sb = persist.tile((128, 2 * S), FP16)
    Vaug_sb = persist.tile((128, KC * 4 * VW), BF16)
    attn_outT_sb = persist.tile((128, 2 * S), FP16)
    owT_sb = persist.tile((128, 2 * D), FP16)

    pA = ctx.enter_context(tc.tile_pool(name="pA", bufs=1))
    pB = ctx.enter_context(tc.tile_pool(name="pB", bufs=1))
    ps = ctx.enter_context(tc.tile_pool(name="ps", bufs=1, space="PSUM"))

    xT_sb = pA.tile((128, DC * S), FP16)
    wqT_sb = pA.tile((128, DC * JG), FP16)
    wkT_sb = pA.tile((128, DC * JG), FP16)
    wvT_sb = pA.tile((128, DC * JG), FP16)
    bq_sb = pA.tile((128, 2), FP32)
    bk_sb = pA.tile((128, 2), FP32)
    bv_bc = pA.tile((128, JG), FP32)
    ones_f32 = pA.tile((128, 64), FP32)

    # ---------------- DMA issues (sync queue; ~600ns per dma_start, so
    # batch each weight tensor into ONE 3D-AP strided descriptor; a
    # host-side pre-shuffle to contiguous DMAs measured ~18us SLOWER).
    # KT jc0 runs first on the PE, so wk's dc0 chunk leads.
    def chunked_w(src, dst, lo_dc=0):
        ap = bass.AP(tensor=src.tensor, offset=src.offset + lo_dc * 128 * JG,
                     ap=[[JG, 128], [128 * JG, DC - lo_dc], [1, JG]])
        nc.sync.dma_start(out=dst[:, lo_dc * JG:DC * JG], in_=ap)

    # x is issued in st-major column groups (one 3D-AP per group over all
    # dc chunks): everything block (0,0) needs first -- KT-st0, QT-qt0 and
    # the early V chunks all read only st0's columns -- lands by ~9us, so
    # the first exp fires ~13us instead of ~35us (head was DMA-bound).
    def x_cols(lo, hi):
        ap = bass.AP(tensor=xT.tensor, offset=xT.offset + lo,
                     ap=[[S, 128], [128 * S, DC], [1, hi - lo]])
        dst = bass.AP(tensor=xT_sb.tensor, offset=xT_sb.offset + lo,
                      ap=[list(xT_sb.ap[0]), [S, DC], [1, hi - lo]])
        nc.sync.dma_start(out=dst, in_=ap)

    nc.sync.dma_start(out=wkT_sb[:, 0:JG], in_=wkT[0:128, :])
    chunked_w(wkT, wkT_sb, lo_dc=1)
    chunked_w(wqT, wqT_sb)
    x_cols(0, 512)
    chunked_w(wvT, wvT_sb)
    x_cols(512, 1024)
    x_cols(1024, 2048)
    bq_ap = bass.AP(tensor=bq.tensor, offset=bq.offset,
                    ap=[[1, 128], [128, 2]])
    nc.sync.dma_start(out=bq_sb[:, 0:2], in_=bq_ap)
    bk_ap = bass.AP(tensor=bk.tensor, offset=bk.offset,
                    ap=[[1, 128], [128, 2]])
    nc.sync.dma_start(out=bk_sb[:, 0:2], in_=bk_ap)
    bv_bcast = bass.AP(tensor=bv.tensor, offset=bv.offset,
                       ap=[[0, 128]] + list(bv.ap))
    nc.gpsimd.dma_start(out=bv_bc, in_=bv_bcast)
    ow_ap = bass.AP(tensor=owT.tensor, offset=owT.offset,
                    ap=[[D, 128], [128 * D, 2], [1, D]])
    nc.sync.dma_start(out=owT_sb[:, 0:2 * D], in_=ow_ap)

    # ones: Vaug's per-head denominator columns + the K=1 broadcast row.
    # memset can't emit bf16-from-float cleanly everywhere; stage fp32 and
    # DVE-copy (converts) into the bf16 tiles. No DMA involved.
    nc.vector.memset(ones_f32, 1.0)
    nc.vector.tensor_copy(Vaug_sb[:, HD::VW], ones_f32)

    # ---------------- pre-attention projections ----------------
    def proj_unit(w_sb, b_sb, dst, jc, st):
        """Generator: one (weight, jc, st) projection chunk, 2 matmuls per
        next(), bias-add folded into the last step."""
        pu = ps.tile((128, 512), FP32, tag="op", bufs=2, name="pu")
        for dc in range(DC):
            nc.tensor.matmul(
                pu,
                w_sb[:, dc * JG + jc * 128:dc * JG + (jc + 1) * 128],
                xT_sb[:, dc * S + st * 512:dc * S + (st + 1) * 512],
                start=(dc == 0), stop=(dc == DC - 1),
            )
            if dc % 2 == 1 and dc < DC - 1:
                yield
        nc.vector.tensor_scalar_add(
            out=dst[:, jc * S + st * 512:jc * S + (st + 1) * 512],
            in0=pu, scalar1=b_sb[:, jc:jc + 1])
        yield

    # Only KT-st0 and QT-qt0 pre-attention (st0's x columns land first);
    # KT st1-3 run inline in block (0,0)'s first kc slots as their x
    # column groups arrive.
    for _ in proj_unit(wkT_sb, bk_sb, KT_sb, 0, 0):
        pass
    for _ in proj_unit(wqT_sb, bq_sb, QT_sb, 0, 0):
        pass

    # V chunk: [s-chunk, j-local] into Vaug (stride 65), single strided
    # bias-add. Only chunks 0-2 are emitted pre-attention; block (0,0)
    # self-feeds chunk kc+3 inside its kc loop (the whole V phase ran
    # serially before attention in v3 and delayed the first exp to 45us
    # while ACT idled).
    def v_chunk(sc, tag):
        psv = ps.tile((128, 512), FP32, tag=tag,
                      bufs=1 if tag.startswith("av") else 2, name="psv")
        pv = psv[:, 0:JG]
        for dc in range(DC):
            nc.tensor.matmul(
                pv,
                xT_sb[:, dc * S + sc * 128:dc * S + (sc + 1) * 128],
                wvT_sb[:, dc * JG:(dc + 1) * JG],
                start=(dc == 0), stop=(dc == DC - 1),
            )
        base = sc * 4 * VW
        va = Vaug_sb[:, base:base + 4 * VW]
        nc.vector.tensor_add(
            out=bass.AP(tensor=va.tensor, offset=va.offset,
                        ap=[list(va.ap[0]), [VW, 4], [1, HD]]),
            in0=bass.AP(tensor=pv.tensor, offset=pv.offset,
                        ap=[list(pv.ap[0]), [HD, 4], [1, HD]]),
            in1=bass.AP(tensor=bv_bc.tensor, offset=bv_bc.offset,
                        ap=[list(bv_bc.ap[0]), [HD, 4], [1, HD]]))

    for sc in range(3):
        v_chunk(sc, ["av0", "av1", "op"][sc])

    # Remaining projections stream into kc slots as PE fill-in. Emission
    # must always precede consumption (the tile framework records deps at
    # emission): QT-jc0-qt1 drains in block (0,0)'s last 4 slots, qt2/qt3
    # early in block (0,1), KT-jc1 well before pair 1.
    fill_q = deque()
    for st in range(1, QT_TILES):
        fill_q.append(proj_unit(wqT_sb, bq_sb, QT_sb, 0, st))
    for st in range(QT_TILES):
        fill_q.append(proj_unit(wkT_sb, bk_sb, KT_sb, 1, st))
    for st in range(QT_TILES):
        fill_q.append(proj_unit(wqT_sb, bq_sb, QT_sb, 1, st))

    def fill_step():
        while fill_q:
            try:
                next(fill_q[0])
                return
            except StopIteration:
                fill_q.popleft()

    def outproj_unit(st, it):
        """Generator: one [128,512] out-projection tile; 2 matmuls then
        cast+DMA, one next() each."""
        po = ps.tile((128, 512), FP32, tag="op", bufs=2, name="po")
        for jc in range(2):
            nc.tensor.matmul(
                po,
                attn_outT_sb[:, jc * S + st * 128:jc * S + st * 128 + 128],
                owT_sb[:, jc * D + it * 512:jc * D + (it + 1) * 512],
                start=(jc == 0), stop=(jc == 1))
        yield
        ost = pB.tile((128, 512), FP16, tag="ost", bufs=4, name="ost")
        nc.vector.tensor_copy(ost, po)
        nc.sync.dma_start(
            out=out[st * 128:(st + 1) * 128, it * 512:(it + 1) * 512],
            in_=ost)
        yield

    def queue_outproj(qt):
        for st in range(4 * qt, 4 * qt + 4):
            for it in range(2):
                fill_q.append(outproj_unit(st, it))

    # ---------------- attention (pair-outer) ----------------
    # Each block's normalize (bc matmuls + recip + muls) is DEFERRED into
    # the next block's first kc slot: emitting it at block end puts the bc
    # matmuls (which wait a ~1.5us DVE copy chain) ahead of the next
    # block's logits in the PE stream and stalls ACT ~2.5us per boundary.
    def make_normalize(avs0, avs1, d0, d1, base, chunked=False):
        def emit():
            # reciprocal of the [1,512] denominator rows on DVE (full-tile
            # base-0 APs as reciprocal_approx_fast requires), then a
            # partition-stride-0 DMA broadcast on the idle gpsimd queue --
            # replaces two K=1 PE matmuls per block (~3.4us of PE total).
            r0 = pB.tile((1, 512), FP32, tag="r0", bufs=2, name="r0")
            nc.vector.reciprocal_approx_fast(r0, d0)
            r1 = pB.tile((1, 512), FP32, tag="r1", bufs=2, name="r1")
            nc.vector.reciprocal_approx_fast(r1, d1)
            rcs0 = pB.tile((128, 512), FP32, tag="rcs", bufs=2, name="rcs0")
            nc.gpsimd.partition_broadcast(rcs0, r0)
            rcs1 = pB.tile((128, 512), FP32, tag="rcs", bufs=2, name="rcs1")
            nc.gpsimd.partition_broadcast(rcs1, r1)
            chunks = ((0, 128, (12,)), (128, 512, (13, 14, 15))) if chunked \
                else ((0, 512, ()),)
            nu = 0
            for lo, hi, sts in chunks:
                nc.vector.tensor_mul(
                    out=attn_outT_sb[0:HD, base + lo:base + hi],
                    in0=avs0[0:HD, lo:hi], in1=rcs0[0:HD, lo:hi])
                nc.vector.tensor_mul(
                    out=attn_outT_sb[HD:128, base + lo:base + hi],
                    in0=avs1[0:HD, lo:hi], in1=rcs1[0:HD, lo:hi])
                # tail out-projection: rotate over 4 PSUM slots (the lg
                # banks are free once the exps are done) and alternate
                # casts between DVE and the now-idle ACT so the tail is
                # matmul-paced, not cast-paced.
                for st in sts:
                    for it in range(2):
                        po = ps.tile((128, 512), FP32,
                                     tag=["op", "lg"][nu % 2], bufs=2,
                                     name="pof")
                        for jc in range(2):
                            nc.tensor.matmul(
                                po,
                                attn_outT_sb[:, jc * S + st * 128:
                                             jc * S + st * 128 + 128],
                                owT_sb[:, jc * D + it * 512:
                                       jc * D + (it + 1) * 512],
                                start=(jc == 0), stop=(jc == 1))
                        ost = pB.tile((128, 512), FP16, tag="ost", bufs=4,
                                      name="ost")
                        if nu % 2 == 0:
                            nc.vector.tensor_copy(ost, po)
                        else:
                            nc.scalar.activation(
                                ost, po, mybir.ActivationFunctionType.Copy)
                        nc.sync.dma_start(
                            out=out[st * 128:(st + 1) * 128,
                                    it * 512:(it + 1) * 512],
                            in_=ost)
                        nu += 1
        return emit

    norm_pend = None
    for pair in range(2):
        for qt in range(QT_TILES):
            av0 = ps.tile((128, 512), FP32, tag="av0", bufs=1, name="av0")
            av1 = ps.tile((128, 512), FP32, tag="av1", bufs=1, name="av1")
            qcol = pair * S + qt * 512
            pend = None

            def emit_av(kc, at):
                for h, avp, off in ((2 * pair, av0, 0),
                                    (2 * pair + 1, av1, 512)):
                    nc.tensor.matmul(
                        avp[0:VW, :],
                        Vaug_sb[:, kc * 4 * VW + h * VW:
                                kc * 4 * VW + (h + 1) * VW],
                        at[:, off:off + 512],
                        start=(kc == 0), stop=(kc == KC - 1))

            # kc loop, software-pipelined one stage: fill-in matmuls go
            # between the logits matmuls (which never stall) and the AV
            # matmuls for kc-1 (which wait on the exp).
            for kc in range(KC):
                lg = ps.tile((128, 1024), FP32, tag="lg", bufs=2, name="lg")
                kcol = pair * S + kc * 128
                nc.tensor.matmul(
                    lg[:, 0:512],
                    KT_sb[0:64, kcol:kcol + 128],
                    QT_sb[0:64, qcol:qcol + 512],
                    start=True, stop=True, tile_position=(0, 0))
                nc.tensor.matmul(
                    lg[:, 512:1024],
                    KT_sb[64:128, kcol:kcol + 128],
                    QT_sb[64:128, qcol:qcol + 512],
                    start=True, stop=True, tile_position=(64, 0))
                if pair == 0 and qt == 0:
                    # block (0,0) self-feeds: KT st1-3 in the first slots
                    # (their x column groups land just in time), then V
                    # chunk kc just-in-time for next iteration's AV; the
                    # last slots also drain the QT-jc0-qt1 fill unit.
                    if kc < 3:
                        for _ in proj_unit(wkT_sb, bk_sb, KT_sb, 0, kc + 1):
                            pass
                    else:
                        v_chunk(kc, "op")
                    if kc >= KC - 4:
                        fill_step()
                else:
                    fill_step()
                if pend is not None:
                    emit_av(*pend)
                at = pB.tile((128, 1024), BF16, tag="at", bufs=3, name="at")
                nc.scalar.activation(at, lg, EXP)
                pend = (kc, at)
                if kc == 0:
                    if norm_pend is not None:
                        norm_pend()
                        norm_pend = None
                    if pair == 1 and qt > 0:
                        queue_outproj(qt - 1)
            emit_av(*pend)

            # denominator rows first (the bc matmuls need them soonest),
            # then the av rows to SBUF fp32 -- frees the av banks so the
            # next block's first AV matmul doesn't WAR-wait the normalize.
            d0 = pB.tile((1, 512), FP32, tag="d0", bufs=2, name="d0")
            nc.vector.tensor_copy(d0, av0[HD:HD + 1, :])
            d1 = pB.tile((1, 512), FP32, tag="d1", bufs=2, name="d1")
            nc.vector.tensor_copy(d1, av1[HD:HD + 1, :])
            avs0 = pB.tile((VW, 512), FP32, tag="avs0", bufs=2, name="avs0")
            nc.vector.tensor_copy(avs0, av0[0:VW, :])
            avs1 = pB.tile((VW, 512), FP32, tag="avs1", bufs=2, name="avs1")
            nc.vector.tensor_copy(avs1, av1[0:VW, :])
            base = pair * S + qt * 512
            norm_pend = make_normalize(
                avs0, avs1, d0, d1, base,
                chunked=(pair == 1 and qt == QT_TILES - 1))

    # tail: drain leftover fill units, then the last block's normalize with
    # its out-projection st-units interleaved
    while fill_q:
        fill_step()
    norm_pend()


_NC = None


def _build_nc():
    global _NC
    if _NC is not None:
        return _NC
    nc = bacc.Bacc("TRN2", target_bir_lowering=False, debug=False,
                   num_devices=NCORES)
    xT = nc.dram_tensor("xT", [D, S], FP16, kind="ExternalInput").ap()
    wqT = nc.dram_tensor("wqT", [D, JG], FP16, kind="ExternalInput").ap()
    wkT = nc.dram_tensor("wkT", [D, JG], FP16, kind="ExternalInput").ap()
    wvT = nc.dram_tensor("wvT", [D, JG], FP16, kind="ExternalInput").ap()
    bq = nc.dram_tensor("bq", [JG], FP32, kind="ExternalInput").ap()
    bk = nc.dram_tensor("bk", [JG], FP32, kind="ExternalInput").ap()
    bv = nc.dram_tensor("bv", [JG], FP32, kind="ExternalInput").ap()
    owT = nc.dram_tensor("owT", [JG, D], FP16, kind="ExternalInput").ap()
    out = nc.dram_tensor("out", [S, D], FP16, kind="ExternalOutput").ap()
    with tile.TileContext(nc) as tc:
        mha_core_kernel(tc, out, xT, wqT, wkT, wvT, bq, bk, bv, owT)
    nc.compile()
    _NC = nc
    return nc


def _in_maps(x, kqv_w, kqv_b, out_w):
    maps = []
    xT16 = [np.ascontiguousarray(x[b].T.astype(np.float16)) for b in range(B)]
    for c in range(NCORES):
        b, g = divmod(c, 4)
        sl = slice(g * JG, (g + 1) * JG)
        maps.append({
            "xT": xT16[b],
            "wqT": np.ascontiguousarray(kqv_w[0 * D:1 * D][sl].T.astype(np.float16)),
            "wkT": np.ascontiguousarray(kqv_w[1 * D:2 * D][sl].T.astype(np.float16)),
            "wvT": np.ascontiguousarray(kqv_w[2 * D:3 * D][sl].T.astype(np.float16)),
            "bq": np.ascontiguousarray(kqv_b[0 * D:1 * D][sl]),
            "bk": np.ascontiguousarray(kqv_b[1 * D:2 * D][sl]),
            "bv": np.ascontiguousarray(kqv_b[2 * D:3 * D][sl]),
            "owT": np.ascontiguousarray(out_w[:, sl].T.astype(np.float16)),
        })
    return maps


def run_spmd(x, kqv_w, kqv_b, out_w, out_b, trace=False, tmpdir=None):
    nc = _build_nc()
    res = run_bass_kernel_spmd(nc, _in_maps(x, kqv_w, kqv_b, out_w),
                               list(range(NCORES)), tmpdir=tmpdir, trace=trace)
    parts = [np.asarray(res.results[c]["out"], dtype=np.float32)
             for c in range(NCORES)]
    full = np.stack([
        parts[4 * b] + parts[4 * b + 1] + parts[4 * b + 2] + parts[4 * b + 3]
        + out_b[None, :].astype(np.float32)
        for b in range(B)
    ])
    return full, res


def kernel(**inputs):
    x = np.asarray(inputs["x"], dtype=np.float32)
    kqv_w = np.asarray(inputs["kqv_w"], dtype=np.float32)
    kqv_b = np.asarray(inputs["kqv_b"], dtype=np.float32)
    out_w = np.asarray(inputs["out_w"], dtype=np.float32)
    out_b = np.asarray(inputs["out_b"], dtype=np.float32)
    full, _ = run_spmd(x, kqv_w, kqv_b, out_w, out_b)
    return full


# revision 26
# speedup vs baseline: 1.0100x; 1.0100x over previous
"""MultiHeadAttention (B=2, S=2048, D=1024, H=16) on 8 TRN2 NeuronCores.

Sharding: core c -> batch b = c//4, head-group g = c%4 (4 heads = 256 channels).
Each core computes its 4 heads' attention for its batch plus the partial
out-projection (out_w columns for its channel group); host sums the 4 partials
per batch and adds out_b.

v2 design (from NTFF trace analysis of the v1 baseline, 304-362us):
 - At warm clock (2.4 GHz) the attention inner loop is ACT-bound, not
   PE-bound: each [128,512] exp costs (512+352)/1.2 ns -- a 352-cycle fixed
   overhead per ACTIVATE -- and ACT ran 89% busy in the warm stretch while
   PE matmuls have slack. So:
     * exps are batched: ONE ACTIVATE per key-chunk over a [128,1024] PSUM
       tile (two banks, both heads' logits side by side) -> halves the
       per-instruction overhead on the bottleneck engine.
     * phase A is folded into the attention phase: only KT-jc0, V, and
       QT-jc0-qt0 are computed up front (~22us instead of ~49us serial);
       the remaining Q/K projection matmuls stream into the ACT-bound kc
       loop as PE fill-in (2 matmuls per kc slot), loop is pair-outer so
       jc1 projections are only needed after pair 0 completes.
     * out-projection units fill the pair-1 kc slots the same way.
 - Denominators: ones column per head in Vaug (index HD within each VW=65
   group) makes softmax denominators fall out of the AV matmul; each head's
   denominator row is broadcast with its own K=1 ones-matmul (no SBUF->SBUF
   DMA scatter hop), reciprocal on DVE, per-row tensor_mul normalize.
 - av PSUM banks run bufs=1 (8-bank budget: lg 2x2 + av0 + av1 + op 2x2);
   the av rows are copied to SBUF (fp32, full precision) right after the
   last AV matmul so the WAR for the next block clears early.
 - Dtypes: all matmul operands 2-byte (fp16; exp outputs / denominator rows
   bf16 for range -- logits reach ~50 so e^50 overflows fp16). fp32 PE
   matmuls draw the DEC throttle to its lowest p-state; fp8 measured at
   exactly fp16 speed (duty-based clamp, dtype-blind below 2 bytes).
   Accumulation fp32 in PSUM.
 - Timing is thermally sensitive (~60us swings back-to-back); compare runs
   only after >=150s idle.
"""

import os
import sys

import numpy as np

for _p in ("/opt/trn_rl_repo",):
    if os.path.isdir(_p) and _p not in sys.path:
        sys.path.insert(0, _p)

from collections import deque
from contextlib import ExitStack

import concourse.bass as bass
import concourse.tile as tile
from concourse import bacc, mybir
from concourse._compat import with_exitstack
from concourse.bass_utils import run_bass_kernel_spmd

B, S, D = 2, 2048, 1024
H = 16
HD = 64
NCORES = 8
JG = 256          # channels per core (4 heads)
DC = D // 128     # 8 contraction chunks
QT_TILES = 4      # 4 x 512 query tiles
KC = S // 128     # 16 key chunks
VW = 65           # V columns per head incl. ones column
FP32 = mybir.dt.float32
FP16 = mybir.dt.float16
BF16 = mybir.dt.bfloat16
EXP = mybir.ActivationFunctionType.Exp


@with_exitstack
def mha_core_kernel(ctx: ExitStack, tc: tile.TileContext,
                    out, xT, wqT, wkT, wvT, bq, bk, bv, owT):
    nc = tc.nc
    ctx.enter_context(nc.allow_low_precision("2-byte matmul operands"))

    persist = ctx.enter_context(tc.tile_pool(name="persist", bufs=1))
    QT_sb = persist.tile((128, 2 * S), FP16)
    KT_sb = persist.tile((128, 2 * S), FP16)
    Vaug_sb = persist.tile((128, KC * 4 * VW), BF16)
    attn_outT_sb = persist.tile((128, 2 * S), FP16)
    owT_sb = persist.tile((128, 2 * D), FP16)

    pA = ctx.enter_context(tc.tile_pool(name="pA", bufs=1))
    pB = ctx.enter_context(tc.tile_pool(name="pB", bufs=1))
    ps = ctx.enter_context(tc.tile_pool(name="ps", bufs=1, space="PSUM"))

    xT_sb = pA.tile((128, DC * S), FP16)
    wqT_sb = pA.tile((128, DC * JG), FP16)
    wkT_sb = pA.tile((128, DC * JG), FP16)
    wvT_sb = pA.tile((128, DC * JG), FP16)
    bq_sb = pA.tile((128, 2), FP32)
    bk_sb = pA.tile((128, 2), FP32)
    bv_bc = pA.tile((128, JG), FP32)
    ones_f32 = pA.tile((128, 64), FP32)

    # ---------------- DMA issues ----------------
    # x (4MB, fast 2D row-contiguous chunks) streams on the sync queue;
    # ALL weights go on the gpsimd queue so their descriptor generation
    # (~3.4us each for the strided 3D-APs) and wire time run in parallel
    # with x. (A host-side pre-shuffle to contiguous DMAs measured ~18us
    # SLOWER than the strided descriptors -- don't "fix" it.)
    def chunked_w(src, dst, lo_dc=0):
        ap = bass.AP(tensor=src.tensor, offset=src.offset + lo_dc * 128 * JG,
                     ap=[[JG, 128], [128 * JG, DC - lo_dc], [1, JG]])
        nc.gpsimd.dma_start(out=dst[:, lo_dc * JG:DC * JG], in_=ap)

    nc.gpsimd.dma_start(out=wkT_sb[:, 0:JG], in_=wkT[0:128, :])
    for st in range(QT_TILES):
        nc.sync.dma_start(
            out=xT_sb[:, st * 512:(st + 1) * 512],
            in_=xT[0:128, st * 512:(st + 1) * 512])
    chunked_w(wkT, wkT_sb, lo_dc=1)
    for dc in range(1, DC):
        nc.sync.dma_start(out=xT_sb[:, dc * S:(dc + 1) * S],
                          in_=xT[dc * 128:(dc + 1) * 128, :])
    chunked_w(wqT, wqT_sb)
    chunked_w(wvT, wvT_sb)
    bq_ap = bass.AP(tensor=bq.tensor, offset=bq.offset,
                    ap=[[1, 128], [128, 2]])
    nc.gpsimd.dma_start(out=bq_sb[:, 0:2], in_=bq_ap)
    bk_ap = bass.AP(tensor=bk.tensor, offset=bk.offset,
                    ap=[[1, 128], [128, 2]])
    nc.gpsimd.dma_start(out=bk_sb[:, 0:2], in_=bk_ap)
    bv_bcast = bass.AP(tensor=bv.tensor, offset=bv.offset,
                       ap=[[0, 128]] + list(bv.ap))
    nc.gpsimd.dma_start(out=bv_bc, in_=bv_bcast)
    ow_ap = bass.AP(tensor=owT.tensor, offset=owT.offset,
                    ap=[[D, 128], [128 * D, 2], [1, D]])
    nc.gpsimd.dma_start(out=owT_sb[:, 0:2 * D], in_=ow_ap)

    # ones: Vaug's per-head denominator columns + the K=1 broadcast row.
    # memset can't emit bf16-from-float cleanly everywhere; stage fp32 and
    # DVE-copy (converts) into the bf16 tiles. No DMA involved.
    nc.vector.memset(ones_f32, 1.0)
    nc.vector.tensor_copy(Vaug_sb[:, HD::VW], ones_f32)

    # ---------------- pre-attention projections ----------------
    def proj_unit(w_sb, b_sb, dst, jc, st):
        """Generator: one (weight, jc, st) projection chunk, 2 matmuls per
        next(), bias-add folded into the last step."""
        pu = ps.tile((128, 512), FP32, tag="op", bufs=2, name="pu")
        for dc in range(DC):
            nc.tensor.matmul(
                pu,
                w_sb[:, dc * JG + jc * 128:dc * JG + (jc + 1) * 128],
                xT_sb[:, dc * S + st * 512:dc * S + (st + 1) * 512],
                start=(dc == 0), stop=(dc == DC - 1),
            )
            if dc % 2 == 1 and dc < DC - 1:
                yield
        nc.vector.tensor_scalar_add(
            out=dst[:, jc * S + st * 512:jc * S + (st + 1) * 512],
            in0=pu, scalar1=b_sb[:, jc:jc + 1])
        yield

    # KT jc0 dc-outer over 4 st-tile PSUM banks: consumes x dc-chunks as
    # they stream in, finishing ~1us after the last chunk lands.
    pss = [ps.tile((128, 512), FP32, tag=["lg", "lg", "op", "op"][st],
                   bufs=2, name=f"kt{st}") for st in range(QT_TILES)]
    for dc in range(DC):
        for st in range(QT_TILES):
            nc.tensor.matmul(
                pss[st],
                wkT_sb[:, dc * JG:dc * JG + 128],
                xT_sb[:, dc * S + st * 512:dc * S + (st + 1) * 512],
                start=(dc == 0), stop=(dc == DC - 1),
            )
            if dc == DC - 1:
                nc.vector.tensor_scalar_add(
                    out=KT_sb[:, st * 512:(st + 1) * 512],
                    in0=pss[st], scalar1=bk_sb[:, 0:1])

    for _ in proj_unit(wqT_sb, bq_sb, QT_sb, 0, 0):
        pass

    # V chunk: [s-chunk, j-local] into Vaug (stride 65), single strided
    # bias-add. Only chunks 0-2 are emitted pre-attention; block (0,0)
    # self-feeds chunk kc+3 inside its kc loop (the whole V phase ran
    # serially before attention in v3 and delayed the first exp to 45us
    # while ACT idled).
    def v_chunk(sc, tag):
        psv = ps.tile((128, 512), FP32, tag=tag,
                      bufs=1 if tag.startswith("av") else 2, name="psv")
        pv = psv[:, 0:JG]
        for dc in range(DC):
            nc.tensor.matmul(
                pv,
                xT_sb[:, dc * S + sc * 128:dc * S + (sc + 1) * 128],
                wvT_sb[:, dc * JG:(dc + 1) * JG],
                start=(dc == 0), stop=(dc == DC - 1),
            )
        base = sc * 4 * VW
        va = Vaug_sb[:, base:base + 4 * VW]
        nc.vector.tensor_add(
            out=bass.AP(tensor=va.tensor, offset=va.offset,
                        ap=[list(va.ap[0]), [VW, 4], [1, HD]]),
            in0=bass.AP(tensor=pv.tensor, offset=pv.offset,
                        ap=[list(pv.ap[0]), [HD, 4], [1, HD]]),
            in1=bass.AP(tensor=bv_bc.tensor, offset=bv_bc.offset,
                        ap=[list(bv_bc.ap[0]), [HD, 4], [1, HD]]))

    v_chunk(0, "av0")

    # Remaining projections stream into kc slots as PE fill-in. Emission
    # must always precede consumption (the tile framework records deps at
    # emission): QT-jc0-qt1 drains in block (0,0)'s last 4 slots, qt2/qt3
    # early in block (0,1), KT-jc1 well before pair 1.
    fill_q = deque()
    for st in range(2, QT_TILES):
        fill_q.append(proj_unit(wqT_sb, bq_sb, QT_sb, 0, st))
    for st in range(QT_TILES):
        fill_q.append(proj_unit(wkT_sb, bk_sb, KT_sb, 1, st))
    for st in range(QT_TILES):
        fill_q.append(proj_unit(wqT_sb, bq_sb, QT_sb, 1, st))

    def fill_step():
        while fill_q:
            try:
                next(fill_q[0])
                return
            except StopIteration:
                fill_q.popleft()

    def outproj_unit(st, it):
        """Generator: one [128,512] out-projection tile; 2 matmuls then
        cast+DMA, one next() each."""
        po = ps.tile((128, 512), FP32, tag="op", bufs=2, name="po")
        for jc in range(2):
            nc.tensor.matmul(
                po,
                attn_outT_sb[:, jc * S + st * 128:jc * S + st * 128 + 128],
                owT_sb[:, jc * D + it * 512:jc * D + (it + 1) * 512],
                start=(jc == 0), stop=(jc == 1))
        yield
        ost = pB.tile((128, 512), FP16, tag="ost", bufs=4, name="ost")
        nc.vector.tensor_copy(ost, po)
        nc.sync.dma_start(
            out=out[st * 128:(st + 1) * 128, it * 512:(it + 1) * 512],
            in_=ost)
        yield

    def queue_outproj(qt):
        for st in range(4 * qt, 4 * qt + 4):
            for it in range(2):
                fill_q.append(outproj_unit(st, it))

    # ---------------- attention (pair-outer) ----------------
    # Each block's normalize (bc matmuls + recip + muls) is DEFERRED into
    # the next block's first kc slot: emitting it at block end puts the bc
    # matmuls (which wait a ~1.5us DVE copy chain) ahead of the next
    # block's logits in the PE stream and stalls ACT ~2.5us per boundary.
    def make_normalize(avs0, avs1, d0, d1, base, chunked=False):
        def emit():
            # reciprocal of the [1,512] denominator rows on DVE (full-tile
            # base-0 APs as reciprocal_approx_fast requires), then a
            # partition-stride-0 DMA broadcast on the idle gpsimd queue --
            # replaces two K=1 PE matmuls per block (~3.4us of PE total).
            r0 = pB.tile((1, 512), FP32, tag="r0", bufs=2, name="r0")
            nc.vector.reciprocal_approx_fast(r0, d0)
            r1 = pB.tile((1, 512), FP32, tag="r1", bufs=2, name="r1")
            nc.vector.reciprocal_approx_fast(r1, d1)
            rcs0 = pB.tile((128, 512), FP32, tag="rcs", bufs=2, name="rcs0")
            nc.gpsimd.partition_broadcast(rcs0, r0)
            rcs1 = pB.tile((128, 512), FP32, tag="rcs", bufs=2, name="rcs1")
            nc.gpsimd.partition_broadcast(rcs1, r1)
            chunks = ((0, 128, (12,)), (128, 512, (13, 14, 15))) if chunked \
                else ((0, 512, ()),)
            nu = 0
            for lo, hi, sts in chunks:
                nc.vector.tensor_mul(
                    out=attn_outT_sb[0:HD, base + lo:base + hi],
                    in0=avs0[0:HD, lo:hi], in1=rcs0[0:HD, lo:hi])
                nc.vector.tensor_mul(
                    out=attn_outT_sb[HD:128, base + lo:base + hi],
                    in0=avs1[0:HD, lo:hi], in1=rcs1[0:HD, lo:hi])
                # tail out-projection: rotate over 4 PSUM slots (the lg
                # banks are free once the exps are done) and alternate
                # casts between DVE and the now-idle ACT so the tail is
                # matmul-paced, not cast-paced.
                for st in sts:
                    for it in range(2):
                        po = ps.tile((128, 512), FP32,
                                     tag=["op", "lg"][nu % 2], bufs=2,
                                     name="pof")
                        for jc in range(2):
                            nc.tensor.matmul(
                                po,
                                attn_outT_sb[:, jc * S + st * 128:
                                             jc * S + st * 128 + 128],
                                owT_sb[:, jc * D + it * 512:
                                       jc * D + (it + 1) * 512],
                                start=(jc == 0), stop=(jc == 1))
                        ost = pB.tile((128, 512), FP16, tag="ost", bufs=4,
                                      name="ost")
                        if nu % 2 == 0:
                            nc.vector.tensor_copy(ost, po)
                        else:
                            nc.scalar.activation(
                                ost, po, mybir.ActivationFunctionType.Copy)
                        nc.sync.dma_start(
                            out=out[st * 128:(st + 1) * 128,
                                    it * 512:(it + 1) * 512],
                            in_=ost)
                        nu += 1
        return emit

    norm_pend = None
    for pair in range(2):
        for qt in range(QT_TILES):
            av0 = ps.tile((128, 512), FP32, tag="av0", bufs=1, name="av0")
            av1 = ps.tile((128, 512), FP32, tag="av1", bufs=1, name="av1")
            qcol = pair * S + qt * 512
            pend = None

            def emit_av(kc, at):
                for h, avp, off in ((2 * pair, av0, 0),
                                    (2 * pair + 1, av1, 512)):
                    nc.tensor.matmul(
                        avp[0:VW, :],
                        Vaug_sb[:, kc * 4 * VW + h * VW:
                                kc * 4 * VW + (h + 1) * VW],
                        at[:, off:off + 512],
                        start=(kc == 0), stop=(kc == KC - 1))

            # kc loop, software-pipelined one stage: fill-in matmuls go
            # between the logits matmuls (which never stall) and the AV
            # matmuls for kc-1 (which wait on the exp).
            for kc in range(KC):
                lg = ps.tile((128, 1024), FP32, tag="lg", bufs=2, name="lg")
                kcol = pair * S + kc * 128
                nc.tensor.matmul(
                    lg[:, 0:512],
                    KT_sb[0:64, kcol:kcol + 128],
                    QT_sb[0:64, qcol:qcol + 512],
                    start=True, stop=True, tile_position=(0, 0))
                nc.tensor.matmul(
                    lg[:, 512:1024],
                    KT_sb[64:128, kcol:kcol + 128],
                    QT_sb[64:128, qcol:qcol + 512],
                    start=True, stop=True, tile_position=(64, 0))
                if pair == 0 and qt == 0:
                    # block (0,0) self-feeds: V chunk kc+1 just-in-time for
                    # the next iteration's AV (only chunk 0 is pre-made).
                    # QT-jc0-qt1 runs whole in the last slot -- a fill
                    # generator here would interleave its 4-slot PSUM
                    # accumulation with the V chunks' op-tag rotation and
                    # get clobbered.
                    if kc < KC - 1:
                        v_chunk(kc + 1, "op")
                    else:
                        for _ in proj_unit(wqT_sb, bq_sb, QT_sb, 0, 1):
                            pass
                else:
                    fill_step()
                if pend is not None:
                    emit_av(*pend)
                at = pB.tile((128, 1024), BF16, tag="at", bufs=3, name="at")
                nc.scalar.activation(at, lg, EXP)
                pend = (kc, at)
                if kc == 0:
                    if norm_pend is not None:
                        norm_pend()
                        norm_pend = None
                    if pair == 1 and qt > 0:
                        queue_outproj(qt - 1)
            emit_av(*pend)

            # denominator rows first (the bc matmuls need them soonest),
            # then the av rows to SBUF fp32 -- frees the av banks so the
            # next block's first AV matmul doesn't WAR-wait the normalize.
            d0 = pB.tile((1, 512), FP32, tag="d0", bufs=2, name="d0")
            nc.vector.tensor_copy(d0, av0[HD:HD + 1, :])
            d1 = pB.tile((1, 512), FP32, tag="d1", bufs=2, name="d1")
            nc.vector.tensor_copy(d1, av1[HD:HD + 1, :])
            avs0 = pB.tile((VW, 512), FP32, tag="avs0", bufs=2, name="avs0")
            nc.vector.tensor_copy(avs0, av0[0:VW, :])
            avs1 = pB.tile((VW, 512), FP32, tag="avs1", bufs=2, name="avs1")
            nc.vector.tensor_copy(avs1, av1[0:VW, :])
            base = pair * S + qt * 512
            norm_pend = make_normalize(
                avs0, avs1, d0, d1, base,
                chunked=(pair == 1 and qt == QT_TILES - 1))

    # tail: drain leftover fill units, then the last block's normalize with
    # its out-projection st-units interleaved
    while fill_q:
        fill_step()
    norm_pend()


_NC = None


def _build_nc():
    global _NC
    if _NC is not None:
        return _NC
    nc = bacc.Bacc("TRN2", target_bir_lowering=False, debug=False,
                   num_devices=NCORES)
    xT = nc.dram_tensor("xT", [D, S], FP16, kind="ExternalInput").ap()
    wqT = nc.dram_tensor("wqT", [D, JG], FP16, kind="ExternalInput").ap()
    wkT = nc.dram_tensor("wkT", [D, JG], FP16, kind="ExternalInput").ap()
    wvT = nc.dram_tensor("wvT", [D, JG], FP16, kind="ExternalInput").ap()
    bq = nc.dram_tensor("bq", [JG], FP32, kind="ExternalInput").ap()
    bk = nc.dram_tensor("bk", [JG], FP32, kind="ExternalInput").ap()
    bv = nc.dram_tensor("bv", [JG], FP32, kind="ExternalInput").ap()
    owT = nc.dram_tensor("owT", [JG, D], FP16, kind="ExternalInput").ap()
    out = nc.dram_tensor("out", [S, D], FP16, kind="ExternalOutput").ap()
    with tile.TileContext(nc) as tc:
        mha_core_kernel(tc, out, xT, wqT, wkT, wvT, bq, bk, bv, owT)
    nc.compile()
    _NC = nc
    return nc


def _in_maps(x, kqv_w, kqv_b, out_w):
    maps = []
    xT16 = [np.ascontiguousarray(x[b].T.astype(np.float16)) for b in range(B)]
    for c in range(NCORES):
        b, g = divmod(c, 4)
        sl = slice(g * JG, (g + 1) * JG)
        maps.append({
            "xT": xT16[b],
            "wqT": np.ascontiguousarray(kqv_w[0 * D:1 * D][sl].T.astype(np.float16)),
            "wkT": np.ascontiguousarray(kqv_w[1 * D:2 * D][sl].T.astype(np.float16)),
            "wvT": np.ascontiguousarray(kqv_w[2 * D:3 * D][sl].T.astype(np.float16)),
            "bq": np.ascontiguousarray(kqv_b[0 * D:1 * D][sl]),
            "bk": np.ascontiguousarray(kqv_b[1 * D:2 * D][sl]),
            "bv": np.ascontiguousarray(kqv_b[2 * D:3 * D][sl]),
            "owT": np.ascontiguousarray(out_w[:, sl].T.astype(np.float16)),
        })
    return maps


def run_spmd(x, kqv_w, kqv_b, out_w, out_b, trace=False, tmpdir=None):
    nc = _build_nc()
    res = run_bass_kernel_spmd(nc, _in_maps(x, kqv_w, kqv_b, out_w),
                               list(range(NCORES)), tmpdir=tmpdir, trace=trace)
    parts = [np.asarray(res.results[c]["out"], dtype=np.float32)
             for c in range(NCORES)]
    full = np.stack([
        parts[4 * b] + parts[4 * b + 1] + parts[4 * b + 2] + parts[4 * b + 3]
        + out_b[None, :].astype(np.float32)
        for b in range(B)
    ])
    return full, res


def kernel(**inputs):
    x = np.asarray(inputs["x"], dtype=np.float32)
    kqv_w = np.asarray(inputs["kqv_w"], dtype=np.float32)
    kqv_b = np.asarray(inputs["kqv_b"], dtype=np.float32)
    out_w = np.asarray(inputs["out_w"], dtype=np.float32)
    out_b = np.asarray(inputs["out_b"], dtype=np.float32)
    full, _ = run_spmd(x, kqv_w, kqv_b, out_w, out_b)
    return full


# revision 28
# speedup vs baseline: 1.0175x; 1.0074x over previous
"""MultiHeadAttention (B=2, S=2048, D=1024, H=16) on 8 TRN2 NeuronCores.

Sharding: core c -> batch b = c//4, head-group g = c%4 (4 heads = 256 channels).
Each core computes its 4 heads' attention for its batch plus the partial
out-projection (out_w columns for its channel group); host sums the 4 partials
per batch and adds out_b.

v2 design (from NTFF trace analysis of the v1 baseline, 304-362us):
 - At warm clock (2.4 GHz) the attention inner loop is ACT-bound, not
   PE-bound: each [128,512] exp costs (512+352)/1.2 ns -- a 352-cycle fixed
   overhead per ACTIVATE -- and ACT ran 89% busy in the warm stretch while
   PE matmuls have slack. So:
     * exps are batched: ONE ACTIVATE per key-chunk over a [128,1024] PSUM
       tile (two banks, both heads' logits side by side) -> halves the
       per-instruction overhead on the bottleneck engine.
     * phase A is folded into the attention phase: only KT-jc0, V, and
       QT-jc0-qt0 are computed up front (~22us instead of ~49us serial);
       the remaining Q/K projection matmuls stream into the ACT-bound kc
       loop as PE fill-in (2 matmuls per kc slot), loop is pair-outer so
       jc1 projections are only needed after pair 0 completes.
     * out-projection units fill the pair-1 kc slots the same way.
 - Denominators: ones column per head in Vaug (index HD within each VW=65
   group) makes softmax denominators fall out of the AV matmul; each head's
   denominator row is broadcast with its own K=1 ones-matmul (no SBUF->SBUF
   DMA scatter hop), reciprocal on DVE, per-row tensor_mul normalize.
 - av PSUM banks run bufs=1 (8-bank budget: lg 2x2 + av0 + av1 + op 2x2);
   the av rows are copied to SBUF (fp32, full precision) right after the
   last AV matmul so the WAR for the next block clears early.
 - Dtypes: all matmul operands 2-byte (fp16; exp outputs / denominator rows
   bf16 for range -- logits reach ~50 so e^50 overflows fp16). fp32 PE
   matmuls draw the DEC throttle to its lowest p-state; fp8 measured at
   exactly fp16 speed (duty-based clamp, dtype-blind below 2 bytes).
   Accumulation fp32 in PSUM.
 - Timing is thermally sensitive (~60us swings back-to-back); compare runs
   only after >=150s idle.
"""

import os
import sys

import numpy as np

for _p in ("/opt/trn_rl_repo",):
    if os.path.isdir(_p) and _p not in sys.path:
        sys.path.insert(0, _p)

from collections import deque
from contextlib import ExitStack

import concourse.bass as bass
import concourse.tile as tile
from concourse import bacc, mybir
from concourse._compat import with_exitstack
from concourse.bass_utils import run_bass_kernel_spmd

B, S, D = 2, 2048, 1024
H = 16
HD = 64
NCORES = 8
JG = 256          # channels per core (4 heads)
DC = D // 128     # 8 contraction chunks
QT_TILES = 4      # 4 x 512 query tiles
KC = S // 128     # 16 key chunks
VW = 65           # V columns per head incl. ones column
FP32 = mybir.dt.float32
FP16 = mybir.dt.float16
BF16 = mybir.dt.bfloat16
EXP = mybir.ActivationFunctionType.Exp


@with_exitstack
def mha_core_kernel(ctx: ExitStack, tc: tile.TileContext,
                    out, xT, wqT, wkT, wvT, bq, bk, bv, owT):
    nc = tc.nc
    ctx.enter_context(nc.allow_low_precision("2-byte matmul operands"))

    persist = ctx.enter_context(tc.tile_pool(name="persist", bufs=1))
    QT_sb = persist.tile((128, 2 * S), FP16)
    KT_sb = persist.tile((128, 2 * S), FP16)
    Vaug_sb = persist.tile((128, KC * 4 * VW), BF16)
    attn_outT_sb = persist.tile((128, 2 * S), FP16)
    owT_sb = persist.tile((128, 2 * D), FP16)

    pA = ctx.enter_context(tc.tile_pool(name="pA", bufs=1))
    pB = ctx.enter_context(tc.tile_pool(name="pB", bufs=1))
    ps = ctx.enter_context(tc.tile_pool(name="ps", bufs=1, space="PSUM"))

    xT_sb = pA.tile((128, DC * S), FP16)
    wqT_sb = pA.tile((128, DC * JG), FP16)
    wkT_sb = pA.tile((128, DC * JG), FP16)
    wvT_sb = pA.tile((128, DC * JG), FP16)
    bq_sb = pA.tile((128, 2), FP32)
    bk_sb = pA.tile((128, 2), FP32)
    bv_bc = pA.tile((128, JG), FP32)
    ones_f32 = pA.tile((128, 64), FP32)

    # ---------------- DMA issues ----------------
    # x (4MB, fast 2D row-contiguous chunks) streams on the sync queue;
    # ALL weights go on the gpsimd queue so their descriptor generation
    # (~3.4us each for the strided 3D-APs) and wire time run in parallel
    # with x. (A host-side pre-shuffle to contiguous DMAs measured ~18us
    # SLOWER than the strided descriptors -- don't "fix" it.)
    def chunked_w(src, dst, lo_dc=0):
        ap = bass.AP(tensor=src.tensor, offset=src.offset + lo_dc * 128 * JG,
                     ap=[[JG, 128], [128 * JG, DC - lo_dc], [1, JG]])
        nc.gpsimd.dma_start(out=dst[:, lo_dc * JG:DC * JG], in_=ap)

    # One DMA queue sustains only ~190 GB/s on these transfers: striping
    # the x chunks across three queues (sync/vector/scalar round-robin)
    # roughly triples the head's effective wire rate, x lands ~16us not
    # ~33us (KT consumes dc-chunks in order at ~0.9us each).
    nc.gpsimd.dma_start(out=wkT_sb[:, 0:JG], in_=wkT[0:128, :])
    for st in range(QT_TILES):
        nc.sync.dma_start(
            out=xT_sb[:, st * 512:(st + 1) * 512],
            in_=xT[0:128, st * 512:(st + 1) * 512])
    chunked_w(wkT, wkT_sb, lo_dc=1)
    xq = [nc.scalar, nc.sync]
    for dc in range(1, DC):
        xq[(dc - 1) % 2].dma_start(out=xT_sb[:, dc * S:(dc + 1) * S],
                                   in_=xT[dc * 128:(dc + 1) * 128, :])
    chunked_w(wqT, wqT_sb)
    chunked_w(wvT, wvT_sb)
    bq_ap = bass.AP(tensor=bq.tensor, offset=bq.offset,
                    ap=[[1, 128], [128, 2]])
    nc.gpsimd.dma_start(out=bq_sb[:, 0:2], in_=bq_ap)
    bk_ap = bass.AP(tensor=bk.tensor, offset=bk.offset,
                    ap=[[1, 128], [128, 2]])
    nc.gpsimd.dma_start(out=bk_sb[:, 0:2], in_=bk_ap)
    bv_bcast = bass.AP(tensor=bv.tensor, offset=bv.offset,
                       ap=[[0, 128]] + list(bv.ap))
    nc.gpsimd.dma_start(out=bv_bc, in_=bv_bcast)
    ow_ap = bass.AP(tensor=owT.tensor, offset=owT.offset,
                    ap=[[D, 128], [128 * D, 2], [1, D]])
    nc.gpsimd.dma_start(out=owT_sb[:, 0:2 * D], in_=ow_ap)

    # ones: Vaug's per-head denominator columns + the K=1 broadcast row.
    # memset can't emit bf16-from-float cleanly everywhere; stage fp32 and
    # DVE-copy (converts) into the bf16 tiles. No DMA involved.
    nc.vector.memset(ones_f32, 1.0)
    nc.vector.tensor_copy(Vaug_sb[:, HD::VW], ones_f32)

    # ---------------- pre-attention projections ----------------
    def proj_unit(w_sb, b_sb, dst, jc, st):
        """Generator: one (weight, jc, st) projection chunk, 2 matmuls per
        next(), bias-add folded into the last step."""
        pu = ps.tile((128, 512), FP32, tag="op", bufs=2, name="pu")
        for dc in range(DC):
            nc.tensor.matmul(
                pu,
                w_sb[:, dc * JG + jc * 128:dc * JG + (jc + 1) * 128],
                xT_sb[:, dc * S + st * 512:dc * S + (st + 1) * 512],
                start=(dc == 0), stop=(dc == DC - 1),
            )
            if dc % 2 == 1 and dc < DC - 1:
                yield
        nc.vector.tensor_scalar_add(
            out=dst[:, jc * S + st * 512:jc * S + (st + 1) * 512],
            in0=pu, scalar1=b_sb[:, jc:jc + 1])
        yield

    # KT jc0 dc-outer over 4 st-tile PSUM banks: consumes x dc-chunks as
    # they stream in, finishing ~1us after the last chunk lands.
    pss = [ps.tile((128, 512), FP32, tag=["lg", "lg", "op", "op"][st],
                   bufs=2, name=f"kt{st}") for st in range(QT_TILES)]
    for dc in range(DC):
        for st in range(QT_TILES):
            nc.tensor.matmul(
                pss[st],
                wkT_sb[:, dc * JG:dc * JG + 128],
                xT_sb[:, dc * S + st * 512:dc * S + (st + 1) * 512],
                start=(dc == 0), stop=(dc == DC - 1),
            )
            if dc == DC - 1:
                nc.vector.tensor_scalar_add(
                    out=KT_sb[:, st * 512:(st + 1) * 512],
                    in0=pss[st], scalar1=bk_sb[:, 0:1])

    for _ in proj_unit(wqT_sb, bq_sb, QT_sb, 0, 0):
        pass

    # V chunk: [s-chunk, j-local] into Vaug (stride 65), single strided
    # bias-add. Only chunks 0-2 are emitted pre-attention; block (0,0)
    # self-feeds chunk kc+3 inside its kc loop (the whole V phase ran
    # serially before attention in v3 and delayed the first exp to 45us
    # while ACT idled).
    def v_chunk(sc, tag):
        psv = ps.tile((128, 512), FP32, tag=tag,
                      bufs=1 if tag.startswith("av") else 2, name="psv")
        pv = psv[:, 0:JG]
        for dc in range(DC):
            nc.tensor.matmul(
                pv,
                xT_sb[:, dc * S + sc * 128:dc * S + (sc + 1) * 128],
                wvT_sb[:, dc * JG:(dc + 1) * JG],
                start=(dc == 0), stop=(dc == DC - 1),
            )
        base = sc * 4 * VW
        va = Vaug_sb[:, base:base + 4 * VW]
        nc.vector.tensor_add(
            out=bass.AP(tensor=va.tensor, offset=va.offset,
                        ap=[list(va.ap[0]), [VW, 4], [1, HD]]),
            in0=bass.AP(tensor=pv.tensor, offset=pv.offset,
                        ap=[list(pv.ap[0]), [HD, 4], [1, HD]]),
            in1=bass.AP(tensor=bv_bc.tensor, offset=bv_bc.offset,
                        ap=[list(bv_bc.ap[0]), [HD, 4], [1, HD]]))

    v_chunk(0, "av0")

    # Remaining projections stream into kc slots as PE fill-in. Emission
    # must always precede consumption (the tile framework records deps at
    # emission): QT-jc0-qt1 drains in block (0,0)'s last 4 slots, qt2/qt3
    # early in block (0,1), KT-jc1 well before pair 1.
    fill_q = deque()
    for st in range(2, QT_TILES):
        fill_q.append(proj_unit(wqT_sb, bq_sb, QT_sb, 0, st))
    for st in range(QT_TILES):
        fill_q.append(proj_unit(wkT_sb, bk_sb, KT_sb, 1, st))
    for st in range(QT_TILES):
        fill_q.append(proj_unit(wqT_sb, bq_sb, QT_sb, 1, st))

    def fill_step():
        while fill_q:
            try:
                next(fill_q[0])
                return
            except StopIteration:
                fill_q.popleft()

    def outproj_unit(st, it):
        """Generator: one [128,512] out-projection tile; 2 matmuls then
        cast+DMA, one next() each."""
        po = ps.tile((128, 512), FP32, tag="op", bufs=2, name="po")
        for jc in range(2):
            nc.tensor.matmul(
                po,
                attn_outT_sb[:, jc * S + st * 128:jc * S + st * 128 + 128],
                owT_sb[:, jc * D + it * 512:jc * D + (it + 1) * 512],
                start=(jc == 0), stop=(jc == 1))
        yield
        ost = pB.tile((128, 512), FP16, tag="ost", bufs=4, name="ost")
        nc.vector.tensor_copy(ost, po)
        nc.sync.dma_start(
            out=out[st * 128:(st + 1) * 128, it * 512:(it + 1) * 512],
            in_=ost)
        yield

    def queue_outproj(qt):
        for st in range(4 * qt, 4 * qt + 4):
            for it in range(2):
                fill_q.append(outproj_unit(st, it))

    # ---------------- attention (pair-outer) ----------------
    # Each block's normalize (bc matmuls + recip + muls) is DEFERRED into
    # the next block's first kc slot: emitting it at block end puts the bc
    # matmuls (which wait a ~1.5us DVE copy chain) ahead of the next
    # block's logits in the PE stream and stalls ACT ~2.5us per boundary.
    def make_normalize(avs0, avs1, d0, d1, base, chunked=False):
        def emit():
            # reciprocal of the [1,512] denominator rows on DVE (full-tile
            # base-0 APs as reciprocal_approx_fast requires), then a
            # partition-stride-0 DMA broadcast on the idle gpsimd queue --
            # replaces two K=1 PE matmuls per block (~3.4us of PE total).
            r0 = pB.tile((1, 512), FP32, tag="r0", bufs=2, name="r0")
            nc.vector.reciprocal_approx_fast(r0, d0)
            r1 = pB.tile((1, 512), FP32, tag="r1", bufs=2, name="r1")
            nc.vector.reciprocal_approx_fast(r1, d1)
            rcs0 = pB.tile((128, 512), FP32, tag="rcs", bufs=2, name="rcs0")
            nc.gpsimd.partition_broadcast(rcs0, r0)
            rcs1 = pB.tile((128, 512), FP32, tag="rcs", bufs=2, name="rcs1")
            nc.gpsimd.partition_broadcast(rcs1, r1)
            chunks = ((0, 128, (12,)), (128, 512, (13, 14, 15))) if chunked \
                else ((0, 512, ()),)
            nu = 0
            for lo, hi, sts in chunks:
                nc.vector.tensor_mul(
                    out=attn_outT_sb[0:HD, base + lo:base + hi],
                    in0=avs0[0:HD, lo:hi], in1=rcs0[0:HD, lo:hi])
                nc.vector.tensor_mul(
                    out=attn_outT_sb[HD:128, base + lo:base + hi],
                    in0=avs1[0:HD, lo:hi], in1=rcs1[0:HD, lo:hi])
                # tail out-projection: rotate over 4 PSUM slots (the lg
                # banks are free once the exps are done) and alternate
                # casts between DVE and the now-idle ACT so the tail is
                # matmul-paced, not cast-paced.
                for st in sts:
                    for it in range(2):
                        po = ps.tile((128, 512), FP32,
                                     tag=["op", "lg"][nu % 2], bufs=2,
                                     name="pof")
                        for jc in range(2):
                            nc.tensor.matmul(
                                po,
                                attn_outT_sb[:, jc * S + st * 128:
                                             jc * S + st * 128 + 128],
                                owT_sb[:, jc * D + it * 512:
                                       jc * D + (it + 1) * 512],
                                start=(jc == 0), stop=(jc == 1))
                        ost = pB.tile((128, 512), FP16, tag="ost", bufs=4,
                                      name="ost")
                        if nu % 2 == 0:
                            nc.vector.tensor_copy(ost, po)
                        else:
                            nc.scalar.activation(
                                ost, po, mybir.ActivationFunctionType.Copy)
                        nc.sync.dma_start(
                            out=out[st * 128:(st + 1) * 128,
                                    it * 512:(it + 1) * 512],
                            in_=ost)
                        nu += 1
        return emit

    norm_pend = None
    for pair in range(2):
        for qt in range(QT_TILES):
            av0 = ps.tile((128, 512), FP32, tag="av0", bufs=1, name="av0")
            av1 = ps.tile((128, 512), FP32, tag="av1", bufs=1, name="av1")
            qcol = pair * S + qt * 512
            pend = None

            def emit_av(kc, at):
                for h, avp, off in ((2 * pair, av0, 0),
                                    (2 * pair + 1, av1, 512)):
                    nc.tensor.matmul(
                        avp[0:VW, :],
                        Vaug_sb[:, kc * 4 * VW + h * VW:
                                kc * 4 * VW + (h + 1) * VW],
                        at[:, off:off + 512],
                        start=(kc == 0), stop=(kc == KC - 1))

            # kc loop, software-pipelined one stage: fill-in matmuls go
            # between the logits matmuls (which never stall) and the AV
            # matmuls for kc-1 (which wait on the exp).
            for kc in range(KC):
                lg = ps.tile((128, 1024), FP32, tag="lg", bufs=2, name="lg")
                kcol = pair * S + kc * 128
                nc.tensor.matmul(
                    lg[:, 0:512],
                    KT_sb[0:64, kcol:kcol + 128],
                    QT_sb[0:64, qcol:qcol + 512],
                    start=True, stop=True, tile_position=(0, 0))
                nc.tensor.matmul(
                    lg[:, 512:1024],
                    KT_sb[64:128, kcol:kcol + 128],
                    QT_sb[64:128, qcol:qcol + 512],
                    start=True, stop=True, tile_position=(64, 0))
                if pair == 0 and qt == 0:
                    # block (0,0) self-feeds: V chunk kc+1 just-in-time for
                    # the next iteration's AV (only chunk 0 is pre-made).
                    # QT-jc0-qt1 runs whole in the last slot -- a fill
                    # generator here would interleave its 4-slot PSUM
                    # accumulation with the V chunks' op-tag rotation and
                    # get clobbered.
                    if kc < KC - 1:
                        v_chunk(kc + 1, "op")
                    else:
                        for _ in proj_unit(wqT_sb, bq_sb, QT_sb, 0, 1):
                            pass
                else:
                    fill_step()
                if pend is not None:
                    emit_av(*pend)
                at = pB.tile((128, 1024), BF16, tag="at", bufs=3, name="at")
                nc.scalar.activation(at, lg, EXP)
                pend = (kc, at)
                if kc == 0:
                    if norm_pend is not None:
                        norm_pend()
                        norm_pend = None
                    if pair == 1 and qt > 0:
                        queue_outproj(qt - 1)
            emit_av(*pend)

            # denominator rows first (the bc matmuls need them soonest),
            # then the av rows to SBUF fp32 -- frees the av banks so the
            # next block's first AV matmul doesn't WAR-wait the normalize.
            d0 = pB.tile((1, 512), FP32, tag="d0", bufs=2, name="d0")
            nc.vector.tensor_copy(d0, av0[HD:HD + 1, :])
            d1 = pB.tile((1, 512), FP32, tag="d1", bufs=2, name="d1")
            nc.vector.tensor_copy(d1, av1[HD:HD + 1, :])
            avs0 = pB.tile((VW, 512), FP32, tag="avs0", bufs=2, name="avs0")
            nc.vector.tensor_copy(avs0, av0[0:VW, :])
            avs1 = pB.tile((VW, 512), FP32, tag="avs1", bufs=2, name="avs1")
            nc.vector.tensor_copy(avs1, av1[0:VW, :])
            base = pair * S + qt * 512
            norm_pend = make_normalize(
                avs0, avs1, d0, d1, base,
                chunked=(pair == 1 and qt == QT_TILES - 1))

    # tail: drain leftover fill units, then the last block's normalize with
    # its out-projection st-units interleaved
    while fill_q:
        fill_step()
    norm_pend()


_NC = None


def _build_nc():
    global _NC
    if _NC is not None:
        return _NC
    nc = bacc.Bacc("TRN2", target_bir_lowering=False, debug=False,
                   num_devices=NCORES)
    xT = nc.dram_tensor("xT", [D, S], FP16, kind="ExternalInput").ap()
    wqT = nc.dram_tensor("wqT", [D, JG], FP16, kind="ExternalInput").ap()
    wkT = nc.dram_tensor("wkT", [D, JG], FP16, kind="ExternalInput").ap()
    wvT = nc.dram_tensor("wvT", [D, JG], FP16, kind="ExternalInput").ap()
    bq = nc.dram_tensor("bq", [JG], FP32, kind="ExternalInput").ap()
    bk = nc.dram_tensor("bk", [JG], FP32, kind="ExternalInput").ap()
    bv = nc.dram_tensor("bv", [JG], FP32, kind="ExternalInput").ap()
    owT = nc.dram_tensor("owT", [JG, D], FP16, kind="ExternalInput").ap()
    out = nc.dram_tensor("out", [S, D], FP16, kind="ExternalOutput").ap()
    with tile.TileContext(nc) as tc:
        mha_core_kernel(tc, out, xT, wqT, wkT, wvT, bq, bk, bv, owT)
    nc.compile()
    _NC = nc
    return nc


def _in_maps(x, kqv_w, kqv_b, out_w):
    maps = []
    xT16 = [np.ascontiguousarray(x[b].T.astype(np.float16)) for b in range(B)]
    for c in range(NCORES):
        b, g = divmod(c, 4)
        sl = slice(g * JG, (g + 1) * JG)
        maps.append({
            "xT": xT16[b],
            "wqT": np.ascontiguousarray(kqv_w[0 * D:1 * D][sl].T.astype(np.float16)),
            "wkT": np.ascontiguousarray(kqv_w[1 * D:2 * D][sl].T.astype(np.float16)),
            "wvT": np.ascontiguousarray(kqv_w[2 * D:3 * D][sl].T.astype(np.float16)),
            "bq": np.ascontiguousarray(kqv_b[0 * D:1 * D][sl]),
            "bk": np.ascontiguousarray(kqv_b[1 * D:2 * D][sl]),
            "bv": np.ascontiguousarray(kqv_b[2 * D:3 * D][sl]),
            "owT": np.ascontiguousarray(out_w[:, sl].T.astype(np.float16)),
        })
    return maps


def run_spmd(x, kqv_w, kqv_b, out_w, out_b, trace=False, tmpdir=None):
    nc = _build_nc()
    res = run_bass_kernel_spmd(nc, _in_maps(x, kqv_w, kqv_b, out_w),
                               list(range(NCORES)), tmpdir=tmpdir, trace=trace)
    parts = [np.asarray(res.results[c]["out"], dtype=np.float32)
             for c in range(NCORES)]
    full = np.stack([
        parts[4 * b] + parts[4 * b + 1] + parts[4 * b + 2] + parts[4 * b + 3]
        + out_b[None, :].astype(np.float32)
        for b in range(B)
    ])
    return full, res


def kernel(**inputs):
    x = np.asarray(inputs["x"], dtype=np.float32)
    kqv_w = np.asarray(inputs["kqv_w"], dtype=np.float32)
    kqv_b = np.asarray(inputs["kqv_b"], dtype=np.float32)
    out_w = np.asarray(inputs["out_w"], dtype=np.float32)
    out_b = np.asarray(inputs["out_b"], dtype=np.float32)
    full, _ = run_spmd(x, kqv_w, kqv_b, out_w, out_b)
    return full
